# revision 2
# baseline (speedup 1.0000x reference)
"""Bass/Tile kernel for nn_MultiHeadAttention (B=2, S=2048, D=1024, H=16) on 8 trn2 cores.

Sharding: core c -> (b = c//4, head-group hg = c%4). Each core computes 4 heads'
q/k/v projections, relu-attention, and a partial FC (256 of 1024 contraction rows).
Host pre-casts to bf16, pre-transposes x / weight slices, and sums the 4
partials per batch + bias.

v4 design (PE-mode batched phases + paired PSUM evacuation):
  - scores: 2x row-tiled (tile_position (0,0)/(64,0)) concurrent K=64 pairs,
    16-pair phase per (qb, hp); one [P,2,SQ] relu drains each pair
  - attn@v: 2x col-tiled (tile_position (0,0)/(0,64)) concurrent M=64 pairs,
    16-pair chained phase; modes are batched S->A per (qb,hp) so the PE
    mode-switch drain (~110ns) happens ~5x per qb instead of every pair
  - projections/fc run in full 128x128 phases; all PSUM evacuation is paired
    ([P,2,512]-shaped instrs) alternating DVE/ACT weighted ~10:6
  - y output staged to bf16 (halves output DMA); v-projection deferred
    between S(0,0) and A(0,0) so S(0,0) relus overlap full-mode work
"""
import numpy as np
import ml_dtypes

import concourse.bass as bass
import concourse.mybir as mybir
import concourse.tile as tile

F32 = mybir.dt.float32
BF16 = mybir.dt.bfloat16
ts, ds = bass.ts, bass.ds

S = 2048
D = 1024
DL = 256      # per-core q/k/v dim (4 heads x 64)
P = 128
KD = D // P   # 8 k-chunks for projections
SQ = 512      # q-block (matmul N)
NQB = S // SQ # 4
NM = S // P   # 16 kpos chunks
DLC = DL // P # 2


def split_excess_waits(nc, max_embed: int = 1):
    """walrus core_v3 codegen accepts at most one sync-wait per instruction;
    move extra waits onto standalone event-sem instructions inserted before."""
    n_split = 0
    counter = 0
    for f in nc.m.functions:
        for blk in f.blocks:
            insts = blk.instructions
            if not any(
                ins.sync_info is not None and len(ins.sync_info.on_wait) > max_embed
                for ins in insts
            ):
                continue
            newl = []
            for ins in insts:
                si = ins.sync_info
                if si is not None and len(si.on_wait) > max_embed:
                    waits = list(si.on_wait)
                    extra, keep = waits[:-max_embed], waits[-max_embed:]
                    for w in extra:
                        counter += 1
                        es = mybir.InstEventSemaphore(name=f"waitsplit_{counter}")
                        es.engine = ins.engine
                        es.sync_info = mybir.SyncInfo(on_wait=[w], on_update=[])
                        newl.append(es)
                        n_split += 1
                    si.on_wait = keep
                newl.append(ins)
            blk.instructions = newl
    return n_split


def build_nc(with_mask: bool):
    nc = bass.Bass()
    # pre-arranged on host: x[p, c, s] = x.T[128c+p, s]; w[p, c, f] = w.T[128c+p, f]
    xT = nc.dram_tensor("xT", [P, KD, S], BF16, kind="ExternalInput")
    wq = nc.dram_tensor("wq", [P, KD, DL], BF16, kind="ExternalInput")
    wk = nc.dram_tensor("wk", [P, KD, DL], BF16, kind="ExternalInput")
    wv = nc.dram_tensor("wv", [P, KD, DL], BF16, kind="ExternalInput")
    wfc = nc.dram_tensor("wfc", [P, DLC, D], BF16, kind="ExternalInput")
    maskT = nc.dram_tensor("maskT", [S, S], F32, kind="ExternalInput") if with_mask else None
    y = nc.dram_tensor("y", [S, D], BF16, kind="ExternalOutput")

    with tile.TileContext(nc) as tc:
        _Emitter(tc, xT, wq, wk, wv, wfc, maskT, y).run()
    split_excess_waits(nc)
    return nc


# weighted DVE/ACT alternation: DVE is ~1.4x faster on PSUM evac
EVAC_PATTERN = (0, 1, 0, 0, 1, 0, 1, 0, 0, 1, 0, 1, 0, 0, 1, 0)  # 10 DVE : 6 ACT


class _Emitter:
    def __init__(self, tc, xT, wq, wk, wv, wfc, maskT, y):
        self.tc = tc
        self.nc = tc.nc
        self.xT, self.wq, self.wk, self.wv, self.wfc = xT, wq, wk, wv, wfc
        self.maskT, self.y = maskT, y
        self.ev = 0
        self.dq = 0

    # -- engine alternation helpers ----------------------------------------
    def dma(self, out_ap, in_ap):
        eng = (self.nc.sync, self.nc.gpsimd)[self.dq % 2]
        eng.dma_start(out_ap, in_ap)
        self.dq += 1

    def evac(self, out_ap, in_ap, relu: bool):
        """PSUM->SBUF drain, weighted-alternating between DVE and ACT."""
        use_dve = EVAC_PATTERN[self.ev % len(EVAC_PATTERN)] == 0
        self.ev += 1
        if relu:
            if use_dve:
                self.nc.vector.tensor_scalar_max(out_ap, in_ap, 0.0)
            else:
                self.nc.scalar.activation(out_ap, in_ap, mybir.ActivationFunctionType.Relu)
        else:
            if use_dve:
                self.nc.vector.tensor_copy(out_ap, in_ap)
            else:
                self.nc.scalar.copy(out_ap, in_ap)

    # -- emission pieces ----------------------------------------------------
    def kq_pair(self, wsb, dstT, nb):
        """both c-chunks of one projection q-block: 16 MMs -> one paired copyback"""
        nc = self.nc
        pt = self.ps.tile([P, 2, SQ], F32, tag="sc", name=f"pj_{dstT.name}_{nb}")
        for c in range(DLC):
            for k in range(KD):
                nc.tensor.matmul(
                    pt[:, c, :], wsb[:, k, ts(c, P)], self.xb[:, k, ds(nb * SQ, SQ)],
                    start=(k == 0), stop=(k == KD - 1),
                )
        self.evac(dstT[:, :, ds(nb * SQ, SQ)], pt[:, :, :], relu=False)

    def v_pair(self, sp):
        """two kpos-chunks of the v projection: 16 MMs -> one paired copyback"""
        nc = self.nc
        pt = self.ps.tile([P, 2, SQ], F32, tag="sc", name=f"v_{sp}")
        for j in range(2):
            sc = 2 * sp + j
            for k in range(KD):
                nc.tensor.matmul(
                    pt[:, j, ds(0, DL)], self.xb[:, k, ts(sc, P)], self.wv_sb[:, k, :],
                    start=(k == 0), stop=(k == KD - 1),
                )
        self.evac(self.vN[:, ds(2 * sp, 2), :], pt[:, :, ds(0, DL)], relu=False)

    def scores_pair(self, qb, hp, m, attn_t, mtile):
        """2x row-tiled concurrent K=64 matmuls + one paired relu"""
        nc = self.nc
        pt = self.ps.tile([P, 2, SQ], F32, tag="sc", name=f"sc_{qb}_{hp}_{m}")
        for h in range(2):
            nc.tensor.matmul(
                pt[:, h, :],
                self.kT[ds(64 * h, 64), hp, ts(m, P)],
                self.qT[ds(64 * h, 64), hp, ds(qb * SQ, SQ)],
                start=True, stop=True,
                tile_position=(64 * h, 0),
            )
        if mtile is not None:
            for h in range(2):
                nc.vector.tensor_tensor(
                    pt[:, h, :], pt[:, h, :], mtile[:, m, :], mybir.AluOpType.add
                )
        self.evac(attn_t[:, m, :, :], pt[:, :, :], relu=True)

    def av_pair(self, qb, hp, m, attn_t, po):
        """2x col-tiled concurrent M=64 matmuls, chained over m"""
        nc = self.nc
        for h in range(2):
            nc.tensor.matmul(
                po[ds(64 * h, 64), :],
                self.vN[:, m, ds(128 * hp + 64 * h, 64)],
                attn_t[:, m, h, :],
                start=(m == 0), stop=(m == NM - 1),
                tile_position=(0, 64 * h),
            )

    def fc_pair(self, sc):
        """both D-halves of one seq-chunk of fc: 4 MMs -> paired copyback -> DMA"""
        nc = self.nc
        pt = self.ps.tile([P, 2, SQ], F32, tag="sc", name=f"fc_{sc}")
        for eb in range(2):
            for c in range(DLC):
                nc.tensor.matmul(
                    pt[:, eb, :], self.outT[:, c, ts(sc, P)],
                    self.wfc_sb[:, c, ds(eb * SQ, SQ)],
                    start=(c == 0), stop=(c == DLC - 1),
                )
        yt = self.ystage.tile([P, D], BF16, tag="yt", name=f"yt_{sc}")
        self.evac(yt[:].rearrange("p (a b) -> p a b", a=2), pt[:, :, :], relu=False)
        nc.sync.dma_start(self.y[ts(sc, P), :], yt[:])

    def load_mask(self, qb):
        if self.maskT is None:
            return None
        nc = self.nc
        mtile = self.mstg.tile([P, NM, SQ], F32, tag="mask", name=f"mask_{qb}")
        for m in range(NM):
            nc.gpsimd.dma_start(
                mtile[:, m, :],
                self.maskT[:, :].rearrange("(m p) q -> p m q", p=P)[:, m, ds(qb * SQ, SQ)],
            )
        return mtile

    def s_phase(self, qb, hp, mt):
        at = self.attn_pool.tile([P, NM, 2, SQ], BF16, tag="attn", name=f"attn_{qb}_{hp}")
        for m in range(NM):
            self.scores_pair(qb, hp, m, at, mt)
        return at

    def a_phase(self, qb, hp, at):
        po = self.ps.tile([P, SQ], F32, tag="av", bufs=2, name=f"av_{qb}_{hp}")
        for m in range(NM):
            self.av_pair(qb, hp, m, at, po)
        self.evac(self.outT[:, hp, ds(qb * SQ, SQ)], po[:], relu=False)

    # -- main ---------------------------------------------------------------
    def run(self):
        from contextlib import ExitStack

        tc, nc = self.tc, self.nc
        stack = ExitStack()
        sb = stack.enter_context(tc.tile_pool(name="sb", bufs=1))
        # PSUM budget (8 banks): sc-pairs 3x2 banks, av 2x1 bank
        self.ps = stack.enter_context(tc.tile_pool(name="ps", bufs=3, space="PSUM"))
        self.attn_pool = stack.enter_context(tc.tile_pool(name="attn", bufs=2))
        self.mstg = stack.enter_context(tc.tile_pool(name="mstg", bufs=2))
        self.ystage = stack.enter_context(tc.tile_pool(name="ystage", bufs=2))

        self.xb = sb.tile([P, KD, S], BF16, name="xb")
        self.wq_sb = sb.tile([P, KD, DL], BF16, name="wq_sb")
        self.wk_sb = sb.tile([P, KD, DL], BF16, name="wk_sb")
        self.wv_sb = sb.tile([P, KD, DL], BF16, name="wv_sb")
        self.wfc_sb = sb.tile([P, DLC, D], BF16, name="wfc_sb")
        self.qT = sb.tile([P, DLC, S], BF16, name="qT")
        self.kT = sb.tile([P, DLC, S], BF16, name="kT")
        self.vN = sb.tile([P, NM, DL], BF16, name="vN")
        self.outT = sb.tile([P, DLC, S], BF16, name="outT")

        # loads: wq + x-block0 first (q0-proj starts earliest), then wk,
        # remaining x quarters/halves, wv, wfc. weights on gpsimd, x split
        # across sync+gpsimd queues.
        nc.gpsimd.dma_start(self.wq_sb[:], self.wq[:, :, :])
        for k in range(KD):
            nc.sync.dma_start(self.xb[:, k, ds(0, SQ)], self.xT[:, k, ds(0, SQ)])
        nc.gpsimd.dma_start(self.wk_sb[:], self.wk[:, :, :])
        for k in range(KD):
            self.dma(self.xb[:, k, ds(SQ, SQ)], self.xT[:, k, ds(SQ, SQ)])
        for k in range(KD):
            self.dma(self.xb[:, k, ds(S // 2, S // 2)], self.xT[:, k, ds(S // 2, S // 2)])
        nc.gpsimd.dma_start(self.wv_sb[:], self.wv[:, :, :])
        nc.gpsimd.dma_start(self.wfc_sb[:], self.wfc[:, :, :])

        # [full] q-projection for qb0, then all of k
        mt0 = self.load_mask(0)
        self.kq_pair(self.wq_sb, self.qT, 0)  # qT block 0
        for nb in range(NQB):
            self.kq_pair(self.wk_sb, self.kT, nb)

        # qb0: S(0,0) [row] -> v-projection [full] -> A(0,0) [col] -> hp1
        at = self.s_phase(0, 0, mt0)
        for sp in range(NM // 2):
            self.v_pair(sp)
        self.a_phase(0, 0, at)
        at = self.s_phase(0, 1, mt0)
        self.a_phase(0, 1, at)

        for qb in range(1, NQB):
            # [full] q-projection for this qb + fc for the previous qb
            mt = self.load_mask(qb)
            self.kq_pair(self.wq_sb, self.qT, qb)
            for sc in range((qb - 1) * 4, (qb - 1) * 4 + 4):
                self.fc_pair(sc)
            at = self.s_phase(qb, 0, mt)
            self.a_phase(qb, 0, at)
            at = self.s_phase(qb, 1, mt)
            self.a_phase(qb, 1, at)

        for sc in range(12, 16):
            self.fc_pair(sc)

        stack.close()


# ---- host wrapper ---------------------------------------------------------

N_HEAD = 16
_nc_cache = {}


def get_nc(with_mask: bool):
    if with_mask not in _nc_cache:
        _nc_cache[with_mask] = build_nc(with_mask)
    return _nc_cache[with_mask]


def make_in_maps(x, mask, Wq, Wk, Wv, Wfc, with_mask):
    scale = np.float32(1.0 / np.sqrt(D // N_HEAD))
    bf = ml_dtypes.bfloat16
    in_maps = []
    for c in range(8):
        b, hg = divmod(c, 4)
        gs = slice(DL * hg, DL * hg + DL)
        def prearrange(wT, cdim):  # [cdim*128, F] -> [128, cdim, F]
            F = wT.shape[1]
            return np.ascontiguousarray(
                wT.reshape(cdim, P, F).transpose(1, 0, 2)
            ).astype(bf)

        m = {
            "xT": prearrange(x[b].T, KD),
            "wq": prearrange((Wq[gs, :] * scale).T, KD),
            "wk": prearrange(Wk[gs, :].T, KD),
            "wv": prearrange(Wv[gs, :].T, KD),
            "wfc": prearrange(Wfc[:, gs].T, DLC),
        }
        if with_mask:
            m["maskT"] = np.ascontiguousarray(
                np.broadcast_to(mask, (1, 1, S, S))[0, 0].T.astype(np.float32)
            )
        in_maps.append(m)
    return in_maps


def kernel(x, mask, Wq, Wk, Wv, Wfc, bfc):
    """Full-input entry: shards across 8 trn2 cores, returns the full output."""
    from concourse.bass_utils import run_bass_kernel_spmd

    x = np.asarray(x, dtype=np.float32)
    mask = np.asarray(mask, dtype=np.float32)
    Wq = np.asarray(Wq, dtype=np.float32)
    Wk = np.asarray(Wk, dtype=np.float32)
    Wv = np.asarray(Wv, dtype=np.float32)
    Wfc = np.asarray(Wfc, dtype=np.float32)
    bfc = np.asarray(bfc, dtype=np.float32)

    B = x.shape[0]
    with_mask = bool(np.any(mask))
    nc = get_nc(with_mask)
    in_maps = make_in_maps(x, mask, Wq, Wk, Wv, Wfc, with_mask)

    res = run_bass_kernel_spmd(nc, in_maps, core_ids=list(range(8)))
    parts = np.stack([np.asarray(r["y"]) for r in res.results])  # [8, S, D] bf16
    out = parts.astype(np.float64).reshape(B, 4, S, D).sum(axis=1)
    out += bfc.astype(np.float64)
    return out.astype(np.float32)
